# revision 1
# baseline (speedup 1.0000x reference)
"""Trainium2 Bass kernel for nn_CNF: Tsit5 CNF with exact Jacobian-trace.

Strategy (data parallel over 8 NeuronCores, 64 samples each):
  - forward MLP pass in fp32 (exact ReLU masks), transposed layout [feat, sample]
  - exact trace via tangent-basis propagation in bf16:
      trace(J) = sum((((W0y*d0) @ W1 * d1) @ W2) * d2 * W3^T)
    with the last contraction fused into a scalar_tensor_tensor (evict+mask+
    dot+reduce from PSUM in one DVE op per (chunk, sample)).
  - final log-likelihood assembled on host (negligible).
"""
import sys
for _p in ("/opt/trn_rl_repo", "/root/.axon_site/_ro/trn_rl_repo"):
    if _p not in sys.path:
        sys.path.append(_p)

import numpy as np
import ml_dtypes

import concourse.bass as bass
import concourse.tile as tile
from concourse import bacc, mybir

F32 = mybir.dt.float32
BF16 = mybir.dt.bfloat16
AF = mybir.ActivationFunctionType
OP = mybir.AluOpType

D = 128
H = 512
HC = 4
NB = 2
NSTEP = 10
NSTAGE = 6
N_CORES = 8
S = 64                      # samples per core (512 / 8)
T1C = 1.0
DTC = -0.1
TS_C = [0.0, 0.161, 0.327, 0.9, 0.9800255409045097, 1.0]
TS_A = [
    [],
    [0.161],
    [-0.008480655492356989, 0.335480655492357],
    [2.8971530571054935, -6.359448489975075, 4.3622954328695815],
    [5.325864828439257, -11.748883564062828, 7.4955393428898365, -0.09249506636175525],
    [5.86145544294642, -12.92096931784711, 8.159367898576159, -0.071584973281401, -0.028269050394068383],
]
TS_B = [0.09646076681806523, 0.01, 0.4798896504144996, 1.379008574103742, -3.290069515436081, 2.324710524099774]


def _build_nc():
    NG = 4
    SG = S // NG
    GB = SG * D // 512
    SPB = 512 // D

    nc = bacc.Bacc("TRN2", target_bir_lowering=False, debug=False)

    y_in = nc.dram_tensor("y_in", [128, S], F32, kind="ExternalInput")
    dram = {}
    for b in range(NB):
        dram[f"w0y{b}"] = nc.dram_tensor(f"w0y{b}", [128, H], F32, kind="ExternalInput")
        dram[f"w0yT{b}"] = nc.dram_tensor(f"w0yT{b}", [128, HC, D], BF16, kind="ExternalInput")
        dram[f"w1{b}"] = nc.dram_tensor(f"w1{b}", [128, HC, H], BF16, kind="ExternalInput")
        dram[f"w1f{b}"] = nc.dram_tensor(f"w1f{b}", [128, HC, H], F32, kind="ExternalInput")
        dram[f"w2f{b}"] = nc.dram_tensor(f"w2f{b}", [128, HC, H], F32, kind="ExternalInput")
        dram[f"w3f{b}"] = nc.dram_tensor(f"w3f{b}", [128, HC, D], F32, kind="ExternalInput")
        dram[f"w2{b}"] = nc.dram_tensor(f"w2{b}", [128, HC, H], BF16, kind="ExternalInput")
        dram[f"w3{b}"] = nc.dram_tensor(f"w3{b}", [128, HC, D], BF16, kind="ExternalInput")
        dram[f"b0t{b}"] = nc.dram_tensor(f"b0t{b}", [128, NSTEP * 24], F32, kind="ExternalInput")
        dram[f"b1{b}"] = nc.dram_tensor(f"b1{b}", [128, HC], F32, kind="ExternalInput")
        dram[f"b2{b}"] = nc.dram_tensor(f"b2{b}", [128, HC], F32, kind="ExternalInput")
        dram[f"b3{b}"] = nc.dram_tensor(f"b3{b}", [128, 1], F32, kind="ExternalInput")
    y_out = nc.dram_tensor("y_out", [128, S], F32, kind="ExternalOutput")
    lp_out = nc.dram_tensor("lp_out", [1, S], F32, kind="ExternalOutput")

    from contextlib import ExitStack
    with tile.TileContext(nc) as tc, ExitStack() as es:
        cst = es.enter_context(tc.tile_pool(name="cst", bufs=1))
        st = es.enter_context(tc.tile_pool(name="st", bufs=1))
        wk = es.enter_context(tc.tile_pool(name="wk", bufs=2))
        aw = es.enter_context(tc.tile_pool(name="aw", bufs=2))
        t1w = es.enter_context(tc.tile_pool(name="t1w", bufs=2))
        psA = es.enter_context(tc.tile_pool(name="psA", bufs=4, space="PSUM"))
        psB = es.enter_context(tc.tile_pool(name="psB", bufs=3, space="PSUM"))
        psC = es.enter_context(tc.tile_pool(name="psC", bufs=1, space="PSUM"))

        W = {}
        for b in range(NB):
            for nm, shp in (("w0yT", [128, HC, D]), ("w1", [128, HC, H]),
                            ("w2", [128, HC, H]), ("w3", [128, HC, D])):
                t = cst.tile(shp, BF16, name=f"{nm}_{b}")
                nc.sync.dma_start(t[:], dram[f"{nm}{b}"].ap())
                W[(nm, b)] = t
            for nm, shp in (("w0y", [128, H]), ("w1f", [128, HC, H]), ("w2f", [128, HC, H]),
                            ("w3f", [128, HC, D])):
                t = cst.tile(shp, F32, name=f"{nm}_{b}")
                nc.sync.dma_start(t[:], dram[f"{nm}{b}"].ap())
                W[(nm, b)] = t
            for nm, shp in (("b0t", [128, NSTEP * 24]), ("b1", [128, HC]), ("b2", [128, HC]),
                            ("b3", [128, 1])):
                t = cst.tile(shp, F32, name=f"{nm}_{b}")
                nc.sync.dma_start(t[:], dram[f"{nm}{b}"].ap())
                W[(nm, b)] = t
        ones = cst.tile([128, 1], F32, name="ones")
        nc.vector.memset(ones[:], 1.0)

        yT = st.tile([128, S], F32, name="yT")
        nc.sync.dma_start(yT[:], y_in.ap())
        lp = st.tile([1, S], F32, name="lp")
        nc.vector.memset(lp[:], 0.0)
        ky = [st.tile([128, S], F32, name=f"ky{i}") for i in range(NSTAGE)]
        yacc = st.tile([128, S], F32, name="yacc")

        def forward_and_masks(b, yi32, bias0_dyn, d0, d1, d2, h0, h1, h2):
            p0 = psB.tile([128, HC * S], F32, name="pfw", tag="pfw")
            for mc in range(HC):
                nc.tensor.matmul(p0[:, mc * S:(mc + 1) * S],
                                 W[("w0y", b)][:, mc * 128:(mc + 1) * 128],
                                 yi32[:], start=True, stop=True)
            for mc in range(HC):
                nc.vector.tensor_scalar(out=h0[:, mc, :], in0=p0[:, mc * S:(mc + 1) * S],
                                        scalar1=bias0_dyn(mc), scalar2=0.0,
                                        op0=OP.add, op1=OP.max)
            nc.vector.tensor_scalar(out=d0[:], in0=h0[:], scalar1=0.0, scalar2=None, op0=OP.is_gt)
            p1 = psB.tile([128, HC * S], F32, name="pfw", tag="pfw")
            for mc in range(HC):
                for jc in range(HC):
                    nc.tensor.matmul(p1[:, mc * S:(mc + 1) * S],
                                     W[("w1f", b)][:, jc, mc * 128:(mc + 1) * 128],
                                     h0[:, jc, :], start=(jc == 0), stop=(jc == HC - 1))
            for mc in range(HC):
                nc.scalar.activation(h1[:, mc, :], p1[:, mc * S:(mc + 1) * S], AF.Relu,
                                     bias=W[("b1", b)][:, mc:mc + 1], scale=1.0)
            nc.vector.tensor_scalar(out=d1[:], in0=h1[:], scalar1=0.0, scalar2=None, op0=OP.is_gt)
            p2 = psB.tile([128, HC * S], F32, name="pfw", tag="pfw")
            for mc in range(HC):
                for jc in range(HC):
                    nc.tensor.matmul(p2[:, mc * S:(mc + 1) * S],
                                     W[("w2f", b)][:, jc, mc * 128:(mc + 1) * 128],
                                     h1[:, jc, :], start=(jc == 0), stop=(jc == HC - 1))
            for mc in range(HC):
                nc.scalar.activation(h2[:, mc, :], p2[:, mc * S:(mc + 1) * S], AF.Relu,
                                     bias=W[("b2", b)][:, mc:mc + 1], scale=1.0)
            nc.vector.tensor_scalar(out=d2[:], in0=h2[:], scalar1=0.0, scalar2=None, op0=OP.is_gt)
            p3 = psB.tile([128, S], F32, name="pfw", tag="pfw")
            for kc in range(HC):
                nc.tensor.matmul(p3[:], W[("w3f", b)][:, kc, :], h2[:, kc, :],
                                 start=(kc == 0), stop=(kc == HC - 1))
            return p3

        def tangent_trace(b, d0, d1, d2, qout):
            for g in range(NG):
                s0 = g * SG
                A = []
                for kc in range(HC):
                    a = aw.tile([128, SG, D], BF16, name=f"A{kc}")
                    nc.vector.tensor_tensor(
                        a[:],
                        W[("w0yT", b)][:, kc, :].unsqueeze(1).broadcast_to([128, SG, D]),
                        d0[:, kc, s0:s0 + SG].unsqueeze(2).broadcast_to([128, SG, D]),
                        op=OP.mult)
                    A.append(a)
                T1m = []
                for mc in range(HC):
                    t1e = t1w.tile([128, SG * D], BF16, name=f"t1e{mc}")
                    for g2 in range(GB):
                        p = psA.tile([128, 512], F32, name="pmm", tag="pmm")
                        for kc in range(HC):
                            nc.tensor.matmul(p[:], W[("w1", b)][:, kc, mc * 128:(mc + 1) * 128],
                                             A[kc][:].rearrange("p a b -> p (a b)")[:, g2 * 512:(g2 + 1) * 512],
                                             start=(kc == 0), stop=(kc == HC - 1))
                        nc.scalar.activation(t1e[:, g2 * 512:(g2 + 1) * 512], p[:], AF.Copy, scale=1.0)
                    t1m = t1w.tile([128, SG, D], BF16, name=f"t1m{mc}")
                    nc.vector.tensor_tensor(
                        t1m[:], t1e[:].rearrange("p (a b) -> p a b", b=D),
                        d1[:, mc, s0:s0 + SG].unsqueeze(2).broadcast_to([128, SG, D]),
                        op=OP.mult)
                    T1m.append(t1m)
                for m2 in range(HC):
                    for g2 in range(GB):
                        p2 = psA.tile([128, 512], F32, name="pmm", tag="pmm")
                        for jc in range(HC):
                            nc.tensor.matmul(p2[:], W[("w2", b)][:, jc, m2 * 128:(m2 + 1) * 128],
                                             T1m[jc][:].rearrange("p a b -> p (a b)")[:, g2 * 512:(g2 + 1) * 512],
                                             start=(jc == 0), stop=(jc == HC - 1))
                        scr = wk.tile([128, D], F32, name="scr")
                        for sl in range(SPB):
                            s = s0 + g2 * SPB + sl
                            nc.vector.scalar_tensor_tensor(
                                scr[:], p2[:, sl * D:(sl + 1) * D],
                                d2[:, m2, s:s + 1],
                                W[("w3", b)][:, m2, :],
                                OP.mult, OP.mult,
                                accum_out=qout[:, m2, s:s + 1])

        for b in range(NB):
            with tc.For_i(0, NSTEP * 24, 24,
                          hint_engines=(mybir.EngineType.PE, mybir.EngineType.DVE,
                                        mybir.EngineType.Activation)) as stp:
                yi_next = None
                for i in range(NSTAGE):
                    ycur = yT if i == 0 else yi_next

                    h0 = wk.tile([128, HC, S], F32, name="h0")
                    h1 = wk.tile([128, HC, S], F32, name="h1")
                    h2 = wk.tile([128, HC, S], F32, name="h2")
                    d0 = wk.tile([128, HC, S], BF16, name="d0")
                    d1 = wk.tile([128, HC, S], BF16, name="d1")
                    d2 = wk.tile([128, HC, S], BF16, name="d2")

                    def bias0_dyn(mc, _b=b, _i=i):
                        return W[("b0t", _b)][:, bass.ds(stp + (_i * 4 + mc), 1)]

                    pf = forward_and_masks(b, ycur, bias0_dyn, d0, d1, d2, h0, h1, h2)
                    nc.vector.tensor_scalar(out=ky[i][:], in0=pf[:], scalar1=W[("b3", b)][:, 0:1],
                                            scalar2=None, op0=OP.add)

                    if i + 1 < NSTAGE:
                        ii = i + 1
                        ytmp = wk.tile([128, S], F32, name="ytmp")
                        nc.vector.scalar_tensor_tensor(
                            ytmp[:], ky[0][:], float(np.float32(DTC * TS_A[ii][0])), yT[:],
                            OP.mult, OP.add)
                        for j in range(1, ii):
                            c = float(np.float32(DTC * TS_A[ii][j]))
                            if c != 0.0:
                                nc.vector.scalar_tensor_tensor(
                                    ytmp[:], ky[j][:], c, ytmp[:], OP.mult, OP.add)
                        yi_next = ytmp

                    qout = wk.tile([128, HC, S], F32, name="qout")
                    tangent_trace(b, d0, d1, d2, qout)
                    qs = wk.tile([128, S], F32, name="qs")
                    nc.vector.tensor_reduce(qs[:], qout[:].transpose([0, 2, 1]),
                                            mybir.AxisListType.X, OP.add)
                    pk = psC.tile([1, S], F32, name="pk", tag="pk")
                    nc.tensor.matmul(pk[:], ones[:], qs[:], start=True, stop=True)
                    cb = float(np.float32(DTC * TS_B[i]))
                    nc.vector.scalar_tensor_tensor(lp[:], pk[:], cb, lp[:], OP.mult, OP.add)
                    if i == 0:
                        nc.vector.scalar_tensor_tensor(yacc[:], ky[0][:], cb, yT[:], OP.mult, OP.add)
                    else:
                        nc.vector.scalar_tensor_tensor(yacc[:], ky[i][:], cb, yacc[:], OP.mult, OP.add)
                nc.vector.tensor_copy(yT[:], yacc[:])

        nc.sync.dma_start(y_out.ap(), yT[:])
        nc.sync.dma_start(lp_out.ap(), lp[:])

    nc.compile()
    return nc


def _host_prep(inputs):
    y = np.asarray(inputs["y"], np.float32)
    bf = ml_dtypes.bfloat16
    shared = {}
    for b in range(NB):
        W0 = np.asarray(inputs["Ws0"][b], np.float32)
        w0t = W0[0]; W0y = W0[1:]
        W1 = np.asarray(inputs["Ws1"][b], np.float32)
        W2 = np.asarray(inputs["Ws2"][b], np.float32)
        W3 = np.asarray(inputs["Ws3"][b], np.float32)
        b0 = np.asarray(inputs["bs0"][b], np.float32)
        b1 = np.asarray(inputs["bs1"][b], np.float32)
        b2 = np.asarray(inputs["bs2"][b], np.float32)
        b3 = np.asarray(inputs["bs3"][b], np.float32)
        shared[f"w0y{b}"] = W0y.copy()
        shared[f"w1f{b}"] = W1.reshape(HC, 128, H).transpose(1, 0, 2).copy()
        shared[f"w2f{b}"] = W2.reshape(HC, 128, H).transpose(1, 0, 2).copy()
        shared[f"w3f{b}"] = W3.reshape(HC, 128, D).transpose(1, 0, 2).copy()
        shared[f"w0yT{b}"] = np.ascontiguousarray(W0y.T).reshape(HC, 128, D).transpose(1, 0, 2).astype(bf)
        shared[f"w1{b}"] = W1.reshape(HC, 128, H).transpose(1, 0, 2).astype(bf)
        shared[f"w2{b}"] = W2.reshape(HC, 128, H).transpose(1, 0, 2).astype(bf)
        shared[f"w3{b}"] = W3.reshape(HC, 128, D).transpose(1, 0, 2).astype(bf)
        tab = np.zeros((128, NSTEP * 24), np.float32)
        for n in range(NSTEP):
            t = np.float32(T1C) + np.float32(DTC) * np.float32(n)
            for i in range(6):
                ti = t + np.float32(TS_C[i] * DTC)
                v = b0 + ti * w0t
                for c in range(HC):
                    tab[:, n * 24 + i * 4 + c] = v[c * 128:(c + 1) * 128]
        shared[f"b0t{b}"] = tab
        shared[f"b1{b}"] = b1.reshape(HC, 128).T.copy()
        shared[f"b2{b}"] = b2.reshape(HC, 128).T.copy()
        shared[f"b3{b}"] = b3.reshape(128, 1).copy()
    per_core_y = [np.ascontiguousarray(y[c * S:(c + 1) * S].T) for c in range(N_CORES)]
    return shared, per_core_y


_CACHE = {}


def _get_runner():
    if "runner" in _CACHE:
        return _CACHE["runner"]
    import jax
    from jax.sharding import Mesh, PartitionSpec, NamedSharding
    from jax.experimental.shard_map import shard_map
    from concourse import bass2jax

    nc = _build_nc()
    bass2jax.install_neuronx_cc_hook()
    partition_name = nc.partition_id_tensor.name if nc.partition_id_tensor else None
    in_names, out_names, out_avals, zero_outs = [], [], [], []
    for alloc in nc.m.functions[0].allocations:
        if not isinstance(alloc, mybir.MemoryLocationSet):
            continue
        name = alloc.memorylocations[0].name
        if alloc.kind == "ExternalInput":
            if name != partition_name:
                in_names.append(name)
        elif alloc.kind == "ExternalOutput":
            out_names.append(name)
            shape = tuple(alloc.tensor_shape)
            dtype = mybir.dt.np(alloc.dtype)
            out_avals.append(jax.core.ShapedArray(shape, dtype))
            zero_outs.append(np.zeros(shape, dtype))
    n_params = len(in_names)
    n_outs = len(out_avals)
    all_in = in_names + out_names + ([partition_name] if partition_name else [])
    donate = tuple(range(n_params, n_params + n_outs))

    def _body(*args):
        operands = list(args)
        if partition_name is not None:
            operands.append(bass2jax.partition_id_tensor())
        outs = bass2jax._bass_exec_p.bind(
            *operands, out_avals=tuple(out_avals), in_names=tuple(all_in),
            out_names=tuple(out_names), lowering_input_output_aliases=(),
            sim_require_finite=True, sim_require_nnan=True, nc=nc)
        return tuple(outs)

    devices = jax.devices()[:N_CORES]
    mesh = Mesh(np.asarray(devices), ("core",))
    # y is sharded over cores; everything else (weights/bias tables) replicated
    in_specs = tuple(PartitionSpec("core") if nm == "y_in" else PartitionSpec()
                     for nm in in_names) + (PartitionSpec("core"),) * n_outs
    out_specs = (PartitionSpec("core"),) * n_outs
    sharded = jax.jit(
        shard_map(_body, mesh=mesh, in_specs=in_specs, out_specs=out_specs,
                  check_rep=False),
        donate_argnums=donate, keep_unused=True)

    state = dict(jax=jax, mesh=mesh, sharded=sharded, in_names=in_names,
                 out_names=out_names, out_avals=out_avals, zero_outs=zero_outs,
                 NamedSharding=NamedSharding, PartitionSpec=PartitionSpec)
    _CACHE["runner"] = state
    return state


def kernel(**inputs):
    st = _get_runner()
    jax = st["jax"]
    shared, per_core_y = _host_prep(inputs)
    y_cat = np.concatenate(per_core_y, axis=0)          # [8*128, S]
    args = []
    for nm in st["in_names"]:
        if nm == "y_in":
            args.append(y_cat)
        else:
            args.append(shared[nm])
    zeros = [np.zeros((N_CORES * z.shape[0], *z.shape[1:]), z.dtype)
             for z in st["zero_outs"]]
    outs = st["sharded"](*args, *zeros)
    jax.block_until_ready(outs)
    results = {nm: np.asarray(outs[i]).reshape(N_CORES, *st["out_avals"][i].shape)
               for i, nm in enumerate(st["out_names"])}
    ys = np.concatenate([results["y_out"][c].T for c in range(N_CORES)], 0)   # [512, 128]
    lps = np.concatenate([results["lp_out"][c][0] for c in range(N_CORES)], 0)
    log2pi = np.float32(np.log(2.0 * np.pi))
    out = lps + np.float32(-0.5) * (log2pi + np.sum(ys * ys, axis=-1))
    return out.astype(np.float32)
